# revision 1
# baseline (speedup 1.0000x reference)
"""L2 (spectral) contrastive loss on 8 Trainium2 NeuronCores.

Math: with G_x = x.T @ x and G_y = y.T @ y (both [D, D]),
    sum_{i,j} <x_i, y_j>^2 = ||x @ y.T||_F^2 = tr(G_x @ G_y) = sum(G_x * G_y)
so the loss needs only the two Gram matrices (2*N*D^2 MACs) instead of the
[N, N] pairwise product (N^2*D MACs) - a 5.3x FLOP reduction at N=8192, D=768.

Sharding: rows of x and y are split across the 8 cores. Each core computes
partial Grams over its 1024 rows (bf16 matmuls, fp32 PSUM accumulation,
upper-triangle tiles only - Grams are symmetric), plus the diagonal terms
z_i = <x_i, y_i> (fp32). Partials are packed into one fp16 buffer and
combined with a single 8-core AllReduce; every core then redundantly computes
loss = sum(G_x*G_y)/(N*(N-1)) - sum(z^2)/(N*(N-1)) - (2/N)*sum(z)
and core 0's output is returned.
"""
import numpy as np
from contextlib import ExitStack

from concourse import bacc, tile, mybir
from concourse.bass_utils import run_bass_kernel_spmd

N_CORES = 8
N, D = 8192, 768
ROWS = N // N_CORES          # 1024 rows per core
P = 128                      # SBUF partitions
KCH = ROWS // P              # 8 contraction chunks per core
MS = D // P                  # 6 output slabs per Gram

# upper-triangle slab widths and packed column offsets
WIDTHS = [D - P * m for m in range(MS)]              # [768,640,512,384,256,128]
COFF = [sum(WIDTHS[:m]) for m in range(MS)]          # prefix offsets
GCOLS = sum(WIDTHS)                                  # 2688 per Gram
PACK_COLS = 2 * GCOLS + 16                           # + pad/scalar region
SCAL_COL = 2 * GCOLS                                 # scalars at [0, SCAL_COL:+2]

F32 = mybir.dt.float32
F16 = mybir.dt.float16
BF16 = mybir.dt.bfloat16

_CACHE = {}


def _free_chunks(width):
    """Split [0, width) at the 512-column PSUM bank boundary."""
    if width <= 512:
        return [(0, width)]
    return [(0, 512), (512, width)]


def _build():
    nc = bacc.Bacc("TRN2", target_bir_lowering=False, debug=False,
                   num_devices=N_CORES)
    x_ap = nc.dram_tensor("x", [ROWS, D], F32, kind="ExternalInput").ap()
    y_ap = nc.dram_tensor("y", [ROWS, D], F32, kind="ExternalInput").ap()
    loss_ap = nc.dram_tensor("loss", [1, 1], F32, kind="ExternalOutput").ap()

    inv_nn1 = 1.0 / (float(N) * (N - 1))

    with tile.TileContext(nc) as tc:
        with ExitStack() as ctx:
            sb = ctx.enter_context(tc.tile_pool(name="sb", bufs=1))
            ps = ctx.enter_context(tc.tile_pool(name="ps", bufs=1, space="PSUM"))
            dram = ctx.enter_context(tc.tile_pool(name="dram", bufs=1, space="DRAM"))

            # ---- load inputs: [1024, 768] -> [128p, 8k, 768] ----
            xt = sb.tile([P, KCH, D], F32)
            yt = sb.tile([P, KCH, D], F32)
            nc.sync.dma_start(xt[:], x_ap.rearrange("(n p) d -> p n d", p=P))
            nc.sync.dma_start(yt[:], y_ap.rearrange("(n p) d -> p n d", p=P))

            # ---- cast to bf16 for the PE ----
            xb = sb.tile([P, KCH, D], BF16)
            yb = sb.tile([P, KCH, D], BF16)
            for k in range(KCH):
                nc.vector.tensor_copy(xb[:, k, :], xt[:, k, :])
            for k in range(KCH):
                nc.vector.tensor_copy(yb[:, k, :], yt[:, k, :])

            # ---- packed fp16 partials (both Grams + scalars) ----
            pack = sb.tile([P, PACK_COLS], F16)
            nc.vector.memset(pack[:, 2 * GCOLS:PACK_COLS], 0.0)

            # ---- Grams: upper-triangle slabs, bf16 matmul, fp32 PSUM ----
            for gi, src in enumerate((xb, yb)):
                for m in range(MS):
                    w = WIDTHS[m]
                    slab = ps.tile([P, w], F32, tag="slab", bufs=4,
                                   padded_shape=[P, 768])
                    for (c0, c1) in _free_chunks(w):
                        for k in range(KCH):
                            nc.tensor.matmul(
                                slab[:, c0:c1],
                                src[:, k, P * m:P * (m + 1)],
                                src[:, k, P * m + c0:P * m + c1],
                                start=(k == 0),
                                stop=(k == KCH - 1),
                            )
                    off = gi * GCOLS + COFF[m]
                    nc.vector.tensor_copy(pack[:, off:off + w], slab[:, 0:w])

            # ---- diagonal terms z_i = <x_i, y_i> (from bf16, fp32 accum) ----
            zcols = sb.tile([P, KCH], F32)
            zscr = sb.tile([P, D], F32)
            for k in range(KCH):
                nc.vector.scalar_tensor_tensor(
                    zscr[:], xb[:, k, :], 1.0, yb[:, k, :],
                    mybir.AluOpType.mult, mybir.AluOpType.mult,
                    accum_out=zcols[:, k:k + 1],
                )
            zsq = sb.tile([P, KCH], F32)
            nc.vector.tensor_mul(zsq[:], zcols[:], zcols[:])
            zred = sb.tile([P, 2], F32)
            nc.vector.tensor_reduce(zred[:, 0:1], zcols[:], mybir.AxisListType.X,
                                    mybir.AluOpType.add)
            nc.vector.tensor_reduce(zred[:, 1:2], zsq[:], mybir.AxisListType.X,
                                    mybir.AluOpType.add)
            zfin = sb.tile([1, 2], F32)
            nc.gpsimd.tensor_reduce(zfin[:], zred[:], mybir.AxisListType.C,
                                    mybir.AluOpType.add)
            # scale: s1 -> (2/N)*s1, s2 -> s2/(N*(N-1)); quantize to fp16
            zsc = sb.tile([1, 2], F32)
            nc.vector.tensor_scalar_mul(zsc[:, 0:1], zfin[:, 0:1], 2.0 / N)
            nc.vector.tensor_scalar_mul(zsc[:, 1:2], zfin[:, 1:2], inv_nn1)
            nc.vector.tensor_copy(pack[0:1, SCAL_COL:SCAL_COL + 2], zsc[:])

            # ---- single fp16 AllReduce of all partials ----
            cin = dram.tile([P, PACK_COLS], F16)
            cout = dram.tile([P, PACK_COLS], F16, addr_space="Shared")
            nc.sync.dma_start(cin[:], pack[:])
            nc.gpsimd.collective_compute(
                "AllReduce",
                mybir.AluOpType.add,
                replica_groups=[list(range(N_CORES))],
                ins=[cin.opt()],
                outs=[cout.opt()],
            )
            gsum = sb.tile([P, PACK_COLS], F16)
            nc.sync.dma_start(gsum[:], cout[:])

            # ---- dot(G_x, G_y): diag tiles once, strict-upper tiles twice ----
            dscr = sb.tile([P, D], F32)
            dcols = sb.tile([P, 2 * MS - 1], F32)
            di = 0
            for m in range(MS):
                a = COFF[m]
                b = GCOLS + COFF[m]
                nc.vector.scalar_tensor_tensor(
                    dscr[:, 0:P], gsum[:, a:a + P], 1.0, gsum[:, b:b + P],
                    mybir.AluOpType.mult, mybir.AluOpType.mult,
                    accum_out=dcols[:, di:di + 1],
                )
                di += 1
            for m in range(MS - 1):
                w = WIDTHS[m] - P
                a = COFF[m] + P
                b = GCOLS + COFF[m] + P
                nc.vector.scalar_tensor_tensor(
                    dscr[:, 0:w], gsum[:, a:a + w], 1.0, gsum[:, b:b + w],
                    mybir.AluOpType.mult, mybir.AluOpType.mult,
                    accum_out=dcols[:, di:di + 1],
                )
                di += 1
            dred = sb.tile([P, 2], F32)
            nc.vector.tensor_reduce(dred[:, 0:1], dcols[:, 0:MS],
                                    mybir.AxisListType.X, mybir.AluOpType.add)
            nc.vector.tensor_reduce(dred[:, 1:2], dcols[:, MS:2 * MS - 1],
                                    mybir.AxisListType.X, mybir.AluOpType.add)
            dtot = sb.tile([P, 1], F32)
            nc.vector.scalar_tensor_tensor(
                dtot[:], dred[:, 1:2], 2.0, dred[:, 0:1],
                mybir.AluOpType.mult, mybir.AluOpType.add,
            )
            dfin = sb.tile([1, 1], F32)
            nc.gpsimd.tensor_reduce(dfin[:], dtot[:], mybir.AxisListType.C,
                                    mybir.AluOpType.add)

            # ---- loss = dot/(N*(N-1)) - s2_sum - s1_sum ----
            ssum = sb.tile([1, 2], F32)
            nc.vector.tensor_copy(ssum[:], gsum[0:1, SCAL_COL:SCAL_COL + 2])
            res = sb.tile([1, 1], F32)
            nc.vector.tensor_scalar_mul(res[:], dfin[:], inv_nn1)
            nc.vector.tensor_sub(res[:], res[:], ssum[:, 1:2])
            nc.vector.tensor_sub(res[:], res[:], ssum[:, 0:1])
            nc.sync.dma_start(loss_ap[:], res[:])

    nc.compile()
    return nc


def _get_nc():
    if "nc" not in _CACHE:
        _CACHE["nc"] = _build()
    return _CACHE["nc"]


def _run(x, y, trace=False, **trace_kwargs):
    nc = _get_nc()
    x = np.ascontiguousarray(np.asarray(x, dtype=np.float32))
    y = np.ascontiguousarray(np.asarray(y, dtype=np.float32))
    assert x.shape == (N, D) and y.shape == (N, D)
    in_maps = [
        {"x": x[c * ROWS:(c + 1) * ROWS], "y": y[c * ROWS:(c + 1) * ROWS]}
        for c in range(N_CORES)
    ]
    res = run_bass_kernel_spmd(nc, in_maps, list(range(N_CORES)), trace=trace,
                               **trace_kwargs)
    loss = np.float32(res.results[0]["loss"][0, 0])
    return np.asarray(loss, dtype=np.float32).reshape(()), res


def kernel(x, y):
    out, _ = _run(x, y, trace=False)
    return out


# revision 2
# speedup vs baseline: 1.1946x; 1.1946x over previous
"""L2 (spectral) contrastive loss on 8 Trainium2 NeuronCores.

Math: with G_x = x.T @ x and G_y = y.T @ y (both [D, D]),
    sum_{i,j} <x_i, y_j>^2 = ||x @ y.T||_F^2 = tr(G_x @ G_y) = sum(G_x * G_y)
so the loss needs only the two Gram matrices (2*N*D^2 MACs) instead of the
[N, N] pairwise product (N^2*D MACs) - a 5.3x FLOP reduction at N=8192, D=768.

Sharding: rows of x and y are split across the 8 cores. Each core computes
partial Grams over its 1024 rows (bf16 matmuls, fp32 PSUM accumulation,
upper-triangle tiles only - Grams are symmetric), plus the diagonal terms
z_i = <x_i, y_i> (fp32). Partials are packed into one fp16 buffer and
combined with a single 8-core AllReduce; every core then redundantly computes
loss = sum(G_x*G_y)/(N*(N-1)) - sum(z^2)/(N*(N-1)) - (2/N)*sum(z)
and core 0's output is returned.
"""
import numpy as np
from contextlib import ExitStack

from concourse import bacc, tile, mybir
from concourse.bass_utils import run_bass_kernel_spmd

N_CORES = 8
N, D = 8192, 768
ROWS = N // N_CORES          # 1024 rows per core
P = 128                      # SBUF partitions
KCH = ROWS // P              # 8 contraction chunks per core
MS = D // P                  # 6 output slabs per Gram

# upper-triangle slab widths and packed column offsets
WIDTHS = [D - P * m for m in range(MS)]              # [768,640,512,384,256,128]
COFF = [sum(WIDTHS[:m]) for m in range(MS)]          # prefix offsets
GCOLS = sum(WIDTHS)                                  # 2688 per Gram
PACK_COLS = 2 * GCOLS + 16                           # + pad/scalar region
SCAL_COL = 2 * GCOLS                                 # scalars at [0, SCAL_COL:+2]

F32 = mybir.dt.float32
F16 = mybir.dt.float16
BF16 = mybir.dt.bfloat16

_CACHE = {}


def _free_chunks(width):
    """Split [0, width) at the 512-column PSUM bank boundary."""
    if width <= 512:
        return [(0, width)]
    return [(0, 512), (512, width)]


def _build():
    nc = bacc.Bacc("TRN2", target_bir_lowering=False, debug=False,
                   num_devices=N_CORES)
    x_ap = nc.dram_tensor("x", [ROWS, D], F32, kind="ExternalInput").ap()
    y_ap = nc.dram_tensor("y", [ROWS, D], F32, kind="ExternalInput").ap()
    loss_ap = nc.dram_tensor("loss", [1, 1], F32, kind="ExternalOutput").ap()

    inv_nn1 = 1.0 / (float(N) * (N - 1))

    with tile.TileContext(nc) as tc:
        with ExitStack() as ctx:
            sb = ctx.enter_context(tc.tile_pool(name="sb", bufs=1))
            ps = ctx.enter_context(tc.tile_pool(name="ps", bufs=1, space="PSUM"))
            dram = ctx.enter_context(tc.tile_pool(name="dram", bufs=1, space="DRAM"))

            # ---- load inputs: [1024, 768] -> [128p, 8k, 768] ----
            # chunked per k-slice so casts/matmuls start on first-arrival
            xt = sb.tile([P, KCH, D], F32)
            yt = sb.tile([P, KCH, D], F32)
            xr = x_ap.rearrange("(n p) d -> p n d", p=P)
            yr = y_ap.rearrange("(n p) d -> p n d", p=P)
            for k in range(KCH):
                nc.sync.dma_start(xt[:, k, :], xr[:, k, :])
            for k in range(KCH):
                nc.sync.dma_start(yt[:, k, :], yr[:, k, :])

            # ---- cast to bf16 for the PE ----
            xb = sb.tile([P, KCH, D], BF16)
            yb = sb.tile([P, KCH, D], BF16)
            for k in range(KCH):
                nc.vector.tensor_copy(xb[:, k, :], xt[:, k, :])
            for k in range(KCH):
                nc.vector.tensor_copy(yb[:, k, :], yt[:, k, :])

            # ---- packed fp16 partials (both Grams + scalars) ----
            pack = sb.tile([P, PACK_COLS], F16)
            nc.vector.memset(pack[:, 2 * GCOLS:PACK_COLS], 0.0)

            # ---- Grams: upper-triangle slabs, bf16 matmul, fp32 PSUM ----
            for gi, src in enumerate((xb, yb)):
                for m in range(MS):
                    w = WIDTHS[m]
                    slab = ps.tile([P, w], F32, tag="slab", bufs=4,
                                   padded_shape=[P, 768])
                    for (c0, c1) in _free_chunks(w):
                        for k in range(KCH):
                            nc.tensor.matmul(
                                slab[:, c0:c1],
                                src[:, k, P * m:P * (m + 1)],
                                src[:, k, P * m + c0:P * m + c1],
                                start=(k == 0),
                                stop=(k == KCH - 1),
                            )
                    off = gi * GCOLS + COFF[m]
                    nc.vector.tensor_copy(pack[:, off:off + w], slab[:, 0:w])

            # ---- diagonal terms z_i = <x_i, y_i> (from bf16, fp32 accum) ----
            zcols = sb.tile([P, KCH], F32)
            zscr = sb.tile([P, D], F32)
            for k in range(KCH):
                nc.vector.scalar_tensor_tensor(
                    zscr[:], xb[:, k, :], 1.0, yb[:, k, :],
                    mybir.AluOpType.mult, mybir.AluOpType.mult,
                    accum_out=zcols[:, k:k + 1],
                )
            zsq = sb.tile([P, KCH], F32)
            nc.vector.tensor_mul(zsq[:], zcols[:], zcols[:])
            zred = sb.tile([P, 2], F32)
            nc.vector.tensor_reduce(zred[:, 0:1], zcols[:], mybir.AxisListType.X,
                                    mybir.AluOpType.add)
            nc.vector.tensor_reduce(zred[:, 1:2], zsq[:], mybir.AxisListType.X,
                                    mybir.AluOpType.add)
            zfin = sb.tile([1, 2], F32)
            nc.gpsimd.tensor_reduce(zfin[:], zred[:], mybir.AxisListType.C,
                                    mybir.AluOpType.add)
            # scale: s1 -> (2/N)*s1, s2 -> s2/(N*(N-1)); quantize to fp16
            zsc = sb.tile([1, 2], F32)
            nc.vector.tensor_scalar_mul(zsc[:, 0:1], zfin[:, 0:1], 2.0 / N)
            nc.vector.tensor_scalar_mul(zsc[:, 1:2], zfin[:, 1:2], inv_nn1)
            nc.vector.tensor_copy(pack[0:1, SCAL_COL:SCAL_COL + 2], zsc[:])

            # ---- single fp16 AllReduce of all partials ----
            cin = dram.tile([P, PACK_COLS], F16)
            cout = dram.tile([P, PACK_COLS], F16, addr_space="Shared")
            nc.sync.dma_start(cin[:], pack[:])
            nc.gpsimd.collective_compute(
                "AllReduce",
                mybir.AluOpType.add,
                replica_groups=[list(range(N_CORES))],
                ins=[cin.opt()],
                outs=[cout.opt()],
            )
            gsum = sb.tile([P, PACK_COLS], F16)
            nc.sync.dma_start(gsum[:], cout[:])

            # ---- dot(G_x, G_y): diag tiles once, strict-upper tiles twice ----
            dscr = sb.tile([P, D], F32)
            dcols = sb.tile([P, 2 * MS - 1], F32)
            di = 0
            for m in range(MS):
                a = COFF[m]
                b = GCOLS + COFF[m]
                nc.vector.scalar_tensor_tensor(
                    dscr[:, 0:P], gsum[:, a:a + P], 1.0, gsum[:, b:b + P],
                    mybir.AluOpType.mult, mybir.AluOpType.mult,
                    accum_out=dcols[:, di:di + 1],
                )
                di += 1
            for m in range(MS - 1):
                w = WIDTHS[m] - P
                a = COFF[m] + P
                b = GCOLS + COFF[m] + P
                nc.vector.scalar_tensor_tensor(
                    dscr[:, 0:w], gsum[:, a:a + w], 1.0, gsum[:, b:b + w],
                    mybir.AluOpType.mult, mybir.AluOpType.mult,
                    accum_out=dcols[:, di:di + 1],
                )
                di += 1
            dred = sb.tile([P, 2], F32)
            nc.vector.tensor_reduce(dred[:, 0:1], dcols[:, 0:MS],
                                    mybir.AxisListType.X, mybir.AluOpType.add)
            nc.vector.tensor_reduce(dred[:, 1:2], dcols[:, MS:2 * MS - 1],
                                    mybir.AxisListType.X, mybir.AluOpType.add)
            dtot = sb.tile([P, 1], F32)
            nc.vector.scalar_tensor_tensor(
                dtot[:], dred[:, 1:2], 2.0, dred[:, 0:1],
                mybir.AluOpType.mult, mybir.AluOpType.add,
            )
            dfin = sb.tile([1, 1], F32)
            nc.gpsimd.tensor_reduce(dfin[:], dtot[:], mybir.AxisListType.C,
                                    mybir.AluOpType.add)

            # ---- loss = dot/(N*(N-1)) - s2_sum - s1_sum ----
            ssum = sb.tile([1, 2], F32)
            nc.vector.tensor_copy(ssum[:], gsum[0:1, SCAL_COL:SCAL_COL + 2])
            res = sb.tile([1, 1], F32)
            nc.vector.tensor_scalar_mul(res[:], dfin[:], inv_nn1)
            nc.vector.tensor_sub(res[:], res[:], ssum[:, 1:2])
            nc.vector.tensor_sub(res[:], res[:], ssum[:, 0:1])
            nc.sync.dma_start(loss_ap[:], res[:])

    nc.compile()
    return nc


def _get_nc():
    if "nc" not in _CACHE:
        _CACHE["nc"] = _build()
    return _CACHE["nc"]


def _run(x, y, trace=False, **trace_kwargs):
    nc = _get_nc()
    x = np.ascontiguousarray(np.asarray(x, dtype=np.float32))
    y = np.ascontiguousarray(np.asarray(y, dtype=np.float32))
    assert x.shape == (N, D) and y.shape == (N, D)
    in_maps = [
        {"x": x[c * ROWS:(c + 1) * ROWS], "y": y[c * ROWS:(c + 1) * ROWS]}
        for c in range(N_CORES)
    ]
    res = run_bass_kernel_spmd(nc, in_maps, list(range(N_CORES)), trace=trace,
                               **trace_kwargs)
    loss = np.float32(res.results[0]["loss"][0, 0])
    return np.asarray(loss, dtype=np.float32).reshape(()), res


def kernel(x, y):
    out, _ = _run(x, y, trace=False)
    return out


# revision 17
# speedup vs baseline: 1.2758x; 1.0680x over previous
"""L2 (spectral) contrastive loss on 8 Trainium2 NeuronCores.

Math: with G_x = x.T @ x and G_y = y.T @ y (both [D, D]),
    sum_{i,j} <x_i, y_j>^2 = ||x @ y.T||_F^2 = tr(G_x @ G_y) = sum(G_x * G_y)
so the loss needs only the two Gram matrices (2*N*D^2 MACs) instead of the
[N, N] pairwise product (N^2*D MACs) - a 5.3x FLOP reduction at N=8192, D=768.

Sharding: rows of x and y are split across the 8 cores. Each core computes
partial Grams over its 1024 rows (bf16 matmuls, fp32 PSUM accumulation,
upper-triangle tiles only - Grams are symmetric), plus the diagonal terms
z_i = <x_i, y_i> (fp32). Partials are packed into one fp16 buffer [128, 5392].

Cross-core reduction: a 3-round XOR butterfly over remote SBUF-to-SBUF DMA
(relative-dest remote_dma_broadcast; peers ^1, ^2, ^4), which avoids the
~50-80us collective-firmware latency floor. Every core ends with the full
8-core sums and redundantly computes
    loss = sum(G_x*G_y)/(N*(N-1)) - sum(z^2)/(N*(N-1)) - (2/N)*sum(z);
core 0's output is returned.
"""
import numpy as np
from contextlib import ExitStack

from concourse import bacc, tile, mybir
from concourse.bass_utils import run_bass_kernel_spmd

N_CORES = 8
N, D = 8192, 768
ROWS = N // N_CORES          # 1024 rows per core
P = 128                      # SBUF partitions
KCH = ROWS // P              # 8 contraction chunks per core
MS = D // P                  # 6 output slabs per Gram

# upper-triangle slab widths and packed column offsets
WIDTHS = [D - P * m for m in range(MS)]              # [768,640,512,384,256,128]
COFF = [sum(WIDTHS[:m]) for m in range(MS)]          # prefix offsets
GCOLS = sum(WIDTHS)                                  # 2688 per Gram
PACK_COLS = 2 * GCOLS + 16                           # + pad/scalar region
SCAL_COL = 2 * GCOLS                                 # scalars at [0, SCAL_COL:+2]

F32 = mybir.dt.float32
F16 = mybir.dt.float16
BF16 = mybir.dt.bfloat16

# "butterfly": 3-round XOR exchange via remote_dma (no collective firmware).
#   ~40us faster, but intermittently wedges the device (remote SBUF writes
#   racing receiver-side DMA activity) - kept for reference, NOT the default.
# "collective": single fp16 AllReduce through ncfw - reliable.
REDUCE_MODE = "collective"

# butterfly rounds: xor-peer bit, slot list (cross-die dests must sit in
# slots 4-7), and remote-sem increments per received transfer (2 per slot)
BFLY_ROUNDS = [
    (1, [(0, 1)] * 8, 16),
    (2, [(0, 2)] * 8, 16),
    (4, [None] * 4 + [(0, 4)] * 4, 8),
]

_CACHE = {}


def _free_chunks(width):
    """Split [0, width) at the 512-column PSUM bank boundary."""
    if width <= 512:
        return [(0, width)]
    return [(0, 512), (512, width)]


def _build():
    nc = bacc.Bacc("TRN2", target_bir_lowering=False, debug=False,
                   num_devices=N_CORES)
    x_ap = nc.dram_tensor("x", [ROWS, D], F32, kind="ExternalInput").ap()
    y_ap = nc.dram_tensor("y", [ROWS, D], F32, kind="ExternalInput").ap()
    loss_ap = nc.dram_tensor("loss", [1, 1], F32, kind="ExternalOutput").ap()

    inv_nn1 = 1.0 / (float(N) * (N - 1))
    butterfly = REDUCE_MODE == "butterfly"

    if butterfly:
        # epilogue buffers live outside Tile's allocator/dep-tracking
        recvs = [nc.alloc_sbuf_tensor(f"bfly_recv{r}", [P, PACK_COLS], F16).ap()
                 for r in range(3)]
        dscr_r = nc.alloc_sbuf_tensor("dscr_r", [P, 2 * GCOLS - 2 * D + 2 * D], F32).ap()
        dcols_r = nc.alloc_sbuf_tensor("dcols_r", [P, 2 * MS - 1], F32).ap()
        dred_r = nc.alloc_sbuf_tensor("dred_r", [P, 2], F32).ap()
        dtot_r = nc.alloc_sbuf_tensor("dtot_r", [P, 1], F32).ap()
        dfin_r = nc.alloc_sbuf_tensor("dfin_r", [1, 1], F32).ap()
        ssum_r = nc.alloc_sbuf_tensor("ssum_r", [1, 2], F32).ap()
        res_r = nc.alloc_sbuf_tensor("res_r", [1, 1], F32).ap()
        pack_raw = nc.alloc_sbuf_tensor("pack_raw", [P, PACK_COLS], F16).ap()

    with tile.TileContext(nc) as tc:
        with ExitStack() as ctx:
            sb = ctx.enter_context(tc.tile_pool(name="sb", bufs=1))
            ps = ctx.enter_context(tc.tile_pool(name="ps", bufs=1, space="PSUM"))
            dram = ctx.enter_context(tc.tile_pool(name="dram", bufs=1, space="DRAM"))

            # ---- load inputs: [1024, 768] -> [128p, 8k, 768] ----
            # chunked per k-slice so casts/matmuls start on first-arrival
            xt = sb.tile([P, KCH, D], F32)
            yt = sb.tile([P, KCH, D], F32)
            xr = x_ap.rearrange("(n p) d -> p n d", p=P)
            yr = y_ap.rearrange("(n p) d -> p n d", p=P)
            for k in range(KCH):
                nc.sync.dma_start(xt[:, k, :], xr[:, k, :])
            for k in range(KCH):
                nc.sync.dma_start(yt[:, k, :], yr[:, k, :])

            # ---- cast to bf16 for the PE ----
            xb = sb.tile([P, KCH, D], BF16)
            yb = sb.tile([P, KCH, D], BF16)
            for k in range(KCH):
                nc.vector.tensor_copy(xb[:, k, :], xt[:, k, :])
            for k in range(KCH):
                nc.vector.tensor_copy(yb[:, k, :], yt[:, k, :])

            # ---- packed fp16 partials (both Grams + scalars) ----
            pack = pack_raw if butterfly else sb.tile([P, PACK_COLS], F16)
            nc.vector.memset(pack[:, 2 * GCOLS:PACK_COLS], 0.0)
            ones = sb.tile([P, 1], F32)
            nc.vector.memset(ones[:], 1.0)

            # ---- Grams: upper-triangle slabs, bf16 matmul, fp32 PSUM ----
            for gi, src in enumerate((xb, yb)):
                for m in range(MS):
                    w = WIDTHS[m]
                    slab = ps.tile([P, w], F32, tag="slab", bufs=3,
                                   padded_shape=[P, 768])
                    for (c0, c1) in _free_chunks(w):
                        for k in range(KCH):
                            nc.tensor.matmul(
                                slab[:, c0:c1],
                                src[:, k, P * m:P * (m + 1)],
                                src[:, k, P * m + c0:P * m + c1],
                                start=(k == 0),
                                stop=(k == KCH - 1),
                            )
                    off = gi * GCOLS + COFF[m]
                    nc.vector.tensor_copy(pack[:, off:off + w], slab[:, 0:w])

            # ---- diagonal terms z_i = <x_i, y_i> (from bf16, fp32 accum) ----
            zcols = sb.tile([P, KCH], F32)
            zscr = sb.tile([P, D], F32)
            for k in range(KCH):
                nc.vector.scalar_tensor_tensor(
                    zscr[:], xb[:, k, :], 1.0, yb[:, k, :],
                    mybir.AluOpType.mult, mybir.AluOpType.mult,
                    accum_out=zcols[:, k:k + 1],
                )
            zsq = sb.tile([P, KCH], F32)
            nc.vector.tensor_mul(zsq[:], zcols[:], zcols[:])
            zred = sb.tile([P, 2], F32)
            nc.vector.tensor_reduce(zred[:, 0:1], zcols[:], mybir.AxisListType.X,
                                    mybir.AluOpType.add)
            nc.vector.tensor_reduce(zred[:, 1:2], zsq[:], mybir.AxisListType.X,
                                    mybir.AluOpType.add)
            # partition reduction via PE (ones^T @ zred) - gpsimd custom ops
            # (tensor_reduce axis=C) conflict with remote-DMA desc-gen on Q7
            zfin = sb.tile([1, 2], F32)
            pz = ps.tile([1, 2], F32, tag="pz", bufs=1)
            nc.tensor.matmul(pz[0:1, 0:2], ones[:, 0:1], zred[:, 0:2],
                             start=True, stop=True)
            nc.vector.tensor_copy(zfin[:], pz[0:1, 0:2])
            # scale: s1 -> (2/N)*s1, s2 -> s2/(N*(N-1)); quantize to fp16
            zsc = sb.tile([1, 2], F32)
            nc.vector.tensor_scalar_mul(zsc[:, 0:1], zfin[:, 0:1], 2.0 / N)
            nc.vector.tensor_scalar_mul(zsc[:, 1:2], zfin[:, 1:2], inv_nn1)
            nc.vector.tensor_copy(pack[0:1, SCAL_COL:SCAL_COL + 2], zsc[:])

            if not butterfly:
                # ---- single fp16 AllReduce of all partials ----
                cin = dram.tile([P, PACK_COLS], F16)
                cout = dram.tile([P, PACK_COLS], F16, addr_space="Shared")
                nc.sync.dma_start(cin[:], pack[:])
                nc.gpsimd.collective_compute(
                    "AllReduce",
                    mybir.AluOpType.add,
                    replica_groups=[list(range(N_CORES))],
                    ins=[cin.opt()],
                    outs=[cout.opt()],
                )
                gsum = sb.tile([P, PACK_COLS], F16)
                nc.sync.dma_start(gsum[:], cout[:])
                _emit_dot_and_finale(nc, tc, sb, gsum, loss_ap, inv_nn1)

    if butterfly:
        # ---- raw epilogue: butterfly reduce + dot + finale ----
        # (the TileContext-exit drain barrier guarantees `pack` is complete
        # on every core before any send)
        gp, ve, sy = nc.gpsimd, nc.vector, nc.sync
        lsem = nc.alloc_semaphore("bfly_lsem")
        psem = nc.alloc_semaphore("bfly_psem")
        vch = nc.alloc_semaphore("bfly_vch")   # DVE retirement chain
        rsems = [nc.alloc_semaphore(f"bfly_rsem{r}") for r in range(3)]
        nvch = 0

        rounds = BFLY_ROUNDS

        cur = pack_raw
        for r, (bit, rdests, thresh) in enumerate(rounds):
            recv = recvs[r]
            gp.remote_dma_broadcast(
                recv[:], cur[:], remote_sem=rsems[r], local_sem=lsem,
                rdests=rdests,
            ).then_inc(psem, 1)
            gp.wait_ge(psem, r + 1)       # descriptors written
            if r > 0:
                gp.wait_ge(vch, nvch)     # source (prev recv) add retired
            gp.trigger_dma(count=1)
            ve.wait_ge(rsems[r], thresh)  # peer's data fully arrived
            if r > 0:
                ve.wait_ge(vch, nvch)     # prior add retired (RAW on cur)
            ve.tensor_add(recv[:], recv[:], cur[:]).then_inc(vch, 1)
            nvch += 1
            cur = recv

        # dot(G_x, G_y): diag tiles once, strict-upper twice (on vector)
        ve.wait_ge(vch, nvch)             # final add retired before reads
        di = 0
        soff = 0
        last_stt = None
        for m in range(MS):
            a, b = COFF[m], GCOLS + COFF[m]
            last_stt = ve.scalar_tensor_tensor(
                dscr_r[:, soff:soff + P], cur[:, a:a + P], 1.0, cur[:, b:b + P],
                mybir.AluOpType.mult, mybir.AluOpType.mult,
                accum_out=dcols_r[:, di:di + 1],
            )
            di += 1
            soff += P
        for m in range(MS - 1):
            w = WIDTHS[m] - P
            a, b = COFF[m] + P, GCOLS + COFF[m] + P
            last_stt = ve.scalar_tensor_tensor(
                dscr_r[:, soff:soff + w], cur[:, a:a + w], 1.0, cur[:, b:b + w],
                mybir.AluOpType.mult, mybir.AluOpType.mult,
                accum_out=dcols_r[:, di:di + 1],
            )
            di += 1
            soff += w
        last_stt.then_inc(vch, 1)
        nvch += 1
        ve.wait_ge(vch, nvch)             # all dcols retired
        ve.tensor_reduce(dred_r[:, 0:1], dcols_r[:, 0:MS],
                         mybir.AxisListType.X, mybir.AluOpType.add)
        ve.tensor_reduce(dred_r[:, 1:2], dcols_r[:, MS:2 * MS - 1],
                         mybir.AxisListType.X, mybir.AluOpType.add).then_inc(vch, 1)
        nvch += 1
        dsem = nc.alloc_semaphore("bfly_dsem")
        ve.wait_ge(vch, nvch)
        ve.scalar_tensor_tensor(
            dtot_r[:], dred_r[:, 1:2], 2.0, dred_r[:, 0:1],
            mybir.AluOpType.mult, mybir.AluOpType.add,
        ).then_inc(dsem, 1)

        # partition reduction on gpsimd (safe here: all Tile-phase SWDGE
        # desc-gen is quiesced by the context-exit drain)
        gsem = nc.alloc_semaphore("bfly_gsem")
        gp.wait_ge(dsem, 1)
        gp.tensor_reduce(dfin_r[:], dtot_r[:], mybir.AxisListType.C,
                         mybir.AluOpType.add).then_inc(gsem, 1)
        gp.wait_ge(lsem, 16 * len(rounds))  # all sends drained

        fsem = nc.alloc_semaphore("bfly_fsem")
        ve.wait_ge(gsem, 1)
        ve.tensor_copy(ssum_r[:], cur[0:1, SCAL_COL:SCAL_COL + 2]).then_inc(vch, 1)
        nvch += 1
        ve.tensor_scalar_mul(res_r[:], dfin_r[0:1, 0:1], inv_nn1).then_inc(vch, 1)
        nvch += 1
        ve.wait_ge(vch, nvch)
        ve.tensor_sub(res_r[:], res_r[:], ssum_r[:, 1:2]).then_inc(vch, 1)
        nvch += 1
        ve.wait_ge(vch, nvch)
        ve.tensor_sub(res_r[:], res_r[:], ssum_r[:, 0:1]).then_inc(fsem, 1)

        osem = nc.alloc_semaphore("bfly_osem")
        sy.wait_ge(fsem, 1)
        sy.dma_start(loss_ap[:], res_r[:]).then_inc(osem, 16)
        sy.wait_ge(osem, 16)

    nc.compile()
    return nc


def _emit_dot_and_finale(nc, tc, sb, gsum, loss_ap, inv_nn1):
    """Tile-scheduled dot + finale (collective mode only)."""
    dscr = sb.tile([P, D], F32)
    dcols = sb.tile([P, 2 * MS - 1], F32)
    di = 0
    for m in range(MS):
        a, b = COFF[m], GCOLS + COFF[m]
        nc.vector.scalar_tensor_tensor(
            dscr[:, 0:P], gsum[:, a:a + P], 1.0, gsum[:, b:b + P],
            mybir.AluOpType.mult, mybir.AluOpType.mult,
            accum_out=dcols[:, di:di + 1],
        )
        di += 1
    for m in range(MS - 1):
        w = WIDTHS[m] - P
        a, b = COFF[m] + P, GCOLS + COFF[m] + P
        nc.vector.scalar_tensor_tensor(
            dscr[:, 0:w], gsum[:, a:a + w], 1.0, gsum[:, b:b + w],
            mybir.AluOpType.mult, mybir.AluOpType.mult,
            accum_out=dcols[:, di:di + 1],
        )
        di += 1
    dred = sb.tile([P, 2], F32)
    nc.vector.tensor_reduce(dred[:, 0:1], dcols[:, 0:MS],
                            mybir.AxisListType.X, mybir.AluOpType.add)
    nc.vector.tensor_reduce(dred[:, 1:2], dcols[:, MS:2 * MS - 1],
                            mybir.AxisListType.X, mybir.AluOpType.add)
    dtot = sb.tile([P, 1], F32)
    nc.vector.scalar_tensor_tensor(
        dtot[:], dred[:, 1:2], 2.0, dred[:, 0:1],
        mybir.AluOpType.mult, mybir.AluOpType.add,
    )
    dfin = sb.tile([1, 1], F32)
    nc.gpsimd.tensor_reduce(dfin[:], dtot[:], mybir.AxisListType.C,
                            mybir.AluOpType.add)
    ssum = sb.tile([1, 2], F32)
    nc.vector.tensor_copy(ssum[:], gsum[0:1, SCAL_COL:SCAL_COL + 2])
    res = sb.tile([1, 1], F32)
    nc.vector.tensor_scalar_mul(res[:], dfin[:], inv_nn1)
    nc.vector.tensor_sub(res[:], res[:], ssum[:, 1:2])
    nc.vector.tensor_sub(res[:], res[:], ssum[:, 0:1])
    nc.sync.dma_start(loss_ap[:], res[:])


def _get_nc():
    if "nc" not in _CACHE:
        _CACHE["nc"] = _build()
    return _CACHE["nc"]


def _run(x, y, trace=False, **trace_kwargs):
    nc = _get_nc()
    x = np.ascontiguousarray(np.asarray(x, dtype=np.float32))
    y = np.ascontiguousarray(np.asarray(y, dtype=np.float32))
    assert x.shape == (N, D) and y.shape == (N, D)
    in_maps = [
        {"x": x[c * ROWS:(c + 1) * ROWS], "y": y[c * ROWS:(c + 1) * ROWS]}
        for c in range(N_CORES)
    ]
    res = run_bass_kernel_spmd(nc, in_maps, list(range(N_CORES)), trace=trace,
                               **trace_kwargs)
    loss = np.float32(res.results[0]["loss"][0, 0])
    return np.asarray(loss, dtype=np.float32).reshape(()), res


def kernel(x, y):
    out, _ = _run(x, y, trace=False)
    return out
